# revision 8
# baseline (speedup 1.0000x reference)
"""Bass/Trainium2 kernel for BiDirectionalSymplecticLayer.

Reference computation (B=8192, T=64, F=128, STEPS=8, DT=0.1):
    q_mid = x[:, 32, :]; p_mid = q_mid - x[:, 31, :]
    H(s) = sum(tanh(tanh(s@W1+b1)@W2+b2) @ Wout),  s = [q, p]  (2F = 256)
    leapfrog forward 4 steps with dt=+0.1 and backward 4 steps with dt=-0.1
    out = concat([q_b, p_b, q_mid, p_mid, q_f, p_f], axis=-1)   # [B, 768]

Device strategy (data parallel over 8 cores, 1024 samples each):
  * z1-tracking: instead of carrying the state (q,p), carry
    z1 = [q;p]@W1 + b1 in PSUM.  A leapfrog state update
    (q += dt*g_p, p += -0.5dt*g_q with g = v@W1T) maps to
    z1 += v @ M with a host-precomputed fold matrix
        M_AB = dt*(W1T_p@W1_q) - 0.5dt*(W1T_q@W1_p)   (A-evals)
        M_BA = -0.5dt*(W1T_q@W1_p)                    (B-evals)
    applied directly as a PSUM-accumulating matmul (start=False).
    This removes the per-eval L1 matmul, state casts and most of L4.
  * per eval: h1=tanh(z1) [ACT->fp8]; z2=h1@W2 [fp8 DoubleRow];
    h2=tanh(z2) [ACT->bf16]; sq2=h2^2 [DVE->fp8]; pd=sq2@W2TW
    [fp8 DoubleRow]; v=(pd-c2)*(sq1-1)/16 [Pool, ->fp8];
    z1 += v@(16*M) [fp8 DoubleRow].  c2 = colsum(W2TW) folds the
    wout gradient seed; the 16x scaling keeps M out of fp8 subnormals.
  * final states recovered from v-sums: q_f = q0 + 16dt*(SvA@W1T_p),
    p_f = p0 - 8dt*((SvA+SvB)@W1T_q), Sv accumulated in fp32.
  * chains (forward/backward) run sequentially so both z1 tiles +
    working psum fit in the 8 PSUM banks; batch is split into 2
    staggered streams of 512 for cross-stream engine overlap.
  * the first gradient eval is identical for both chains: chain 1
    reuses chain 0's stored v_A1 and skips its eval 0.
"""

import os
import sys

import numpy as np
import ml_dtypes

try:
    import concourse.bass as bass
except ImportError:  # fresh grading dir: fall back to the repo paths
    for p in ("/root/.axon_site", "/root/.axon_site/_ro/trn_rl_repo",
              "/root/.axon_site/_ro/pypackages", "/opt/trn_rl_repo", "/opt/pypackages"):
        if os.path.isdir(p) and p not in sys.path:
            sys.path.append(p)
    import concourse.bass as bass

import concourse.bacc as bacc
import concourse.mybir as mybir
import concourse.tile as tile
from concourse.bass_utils import run_bass_kernel_spmd

F32 = mybir.dt.float32
BF16 = mybir.dt.bfloat16
F8 = mybir.dt.float8e4
ALU = mybir.AluOpType
AF = mybir.ActivationFunctionType
DR = mybir.MatmulPerfMode.DoubleRow

N_CORES = 8
B = 8192
Bc = B // N_CORES          # 1024 samples per core
F = 128                    # feature dim (= partition dim)
MID = 32
DT = 0.1
NS = 2                     # streams per core
Bh = Bc // NS              # 512 samples per stream
NCH = Bh // 256            # 256-wide moving chunks per DoubleRow matmul


def _build_program(with_bias):
    nc = bacc.Bacc()

    # per-core inputs
    qt_d = nc.declare_dram_parameter("qt", [F, Bc], F32, isOutput=False)
    pt_d = nc.declare_dram_parameter("pt", [F, Bc], F32, isOutput=False)
    s0_d = nc.declare_dram_parameter("s0", [F, 2, Bc], BF16, isOutput=False)
    # replicated weights, pre-blocked host-side to [128, kc, m]
    w1_d = nc.declare_dram_parameter("w1", [F, 2, 2 * F], BF16, isOutput=False)
    w2_d = nc.declare_dram_parameter("w2", [F, 2, 2 * F], F8, isOutput=False)
    w2tw_d = nc.declare_dram_parameter("w2tw", [F, 2, 2 * F], F8, isOutput=False)
    w1t_d = nc.declare_dram_parameter("w1t8", [F, 2, 2 * F], F8, isOutput=False)
    c2n_d = nc.declare_dram_parameter("c2n", [2, 2, 2 * F], BF16, isOutput=False)
    mab_d = [nc.declare_dram_parameter(f"mab{c}", [F, 2, 2 * F], F8, isOutput=False)
             for c in range(2)]
    mba_d = [nc.declare_dram_parameter(f"mba{c}", [F, 2, 2 * F], F8, isOutput=False)
             for c in range(2)]
    b1_d = nc.declare_dram_parameter("b1c", [F, 2], F32, isOutput=False)
    b2_d = nc.declare_dram_parameter("b2c", [F, 2], F32, isOutput=False)
    outs_d = {
        k: nc.declare_dram_parameter(k, [F, Bc], F32, isOutput=True)
        for k in ("oqf", "opf", "oqb", "opb")
    }

    with tile.TileContext(nc) as tc:
        with (
            tc.tile_pool(name="consts", bufs=1) as cw,
            tc.tile_pool(name="acts", bufs=2) as ap_,
            tc.tile_pool(name="outs", bufs=2) as op_,
            tc.tile_pool(name="psum", bufs=1, space="PSUM") as pp,
        ):
            w1s = cw.tile([F, 2, 2 * F], BF16, name="w1s")
            w2s = cw.tile([F, 2, 2 * F], F8, name="w2s")
            w2tws = cw.tile([F, 2, 2 * F], F8, name="w2tws")
            w1ts = cw.tile([F, 2, 2 * F], F8, name="w1ts")
            c2ns = cw.tile([2, 2, 2 * F], BF16, name="c2ns")
            ones2 = cw.tile([2, Bh], BF16, name="ones2")
            mabs = [cw.tile([F, 2, 2 * F], F8, name=f"mabs{c}") for c in range(2)]
            mbas = [cw.tile([F, 2, 2 * F], F8, name=f"mbas{c}") for c in range(2)]
            b1s = cw.tile([F, 2], F32, name="b1s")
            b2s = cw.tile([F, 2], F32, name="b2s")
            s0s = cw.tile([F, 2, Bc], BF16, name="s0s")
            q0s = cw.tile([F, Bc], F32, name="q0s")
            p0s = cw.tile([F, Bc], F32, name="p0s")

            nc.sync.dma_start(out=b1s[:], in_=b1_d[:])
            # warm the ACT tanh table at t=0, hidden under the other DMAs
            warm = cw.tile([F, 1], F32, name="warm")
            nc.scalar.activation(warm[:], b1s[:, 0:1], AF.Tanh)
            nc.sync.dma_start(out=c2ns[:], in_=c2n_d[:])
            nc.vector.memset(ones2[:], 1.0)

            nc.sync.dma_start(out=w1s[:], in_=w1_d[:])
            nc.sync.dma_start(out=s0s[:], in_=s0_d[:])
            nc.sync.dma_start(out=w2s[:], in_=w2_d[:])
            nc.sync.dma_start(out=w2tws[:], in_=w2tw_d[:])
            nc.sync.dma_start(out=b2s[:], in_=b2_d[:])
            for c in range(2):
                nc.sync.dma_start(out=mabs[c][:], in_=mab_d[c][:])
                nc.sync.dma_start(out=mbas[c][:], in_=mba_d[c][:])
            nc.sync.dma_start(out=w1ts[:], in_=w1t_d[:])
            nc.sync.dma_start(out=q0s[:], in_=qt_d[:])
            nc.sync.dma_start(out=p0s[:], in_=pt_d[:])

            va1_keep = [None] * NS   # chain0's first-eval v, reused by chain1

            def init_z1(z1, s):
                # z1 = s0 @ W1 (+ b1), bf16 matmuls, one start per psum bank
                sl = slice(s * Bh, (s + 1) * Bh)
                for jc in range(2):
                    for kc in range(2):
                        nc.tensor.matmul(
                            z1[:, jc, :], w1s[:, kc, jc * F:(jc + 1) * F],
                            s0s[:, kc, sl],
                            start=(kc == 0), stop=(kc == 1),
                            skip_group_check=True)
                if with_bias:
                    for jc in range(2):
                        nc.vector.tensor_scalar(
                            z1[:, jc, :], z1[:, jc, :], b1s[:, jc:jc + 1],
                            None, ALU.add)

            def dr_layer(dst, w, rhs, start=True):
                # dst[:, jc, :] (+)= DoubleRow fp8 w.T @ rhs (512-wide moving)
                for jc in range(2):
                    nc.tensor.matmul(
                        dst[:, jc, :],
                        w[:, :, jc * F:(jc + 1) * F],
                        rhs[:, :, :],
                        start=start, stop=True,
                        perf_mode=DR, skip_group_check=True)

            def emit_eval_pair(c, ev, z1s, streams):
                """One gradient eval + transition, stage-interleaved across
                streams so each engine's in-order queue alternates streams."""
                tgs = {s: f"_{c}_{s}_{ev}" for s in streams}
                h1 = {}
                for s in streams:
                    h1[s] = ap_.tile([F, 2, Bh], F8, name=f"h1{tgs[s]}",
                                     tag=f"h1_{s}")
                    nc.scalar.activation(h1[s][:], z1s[s][:], AF.Tanh)
                pz2 = {}
                for s in streams:
                    pz2[s] = pp.tile([F, 2, Bh], F32, name=f"pz2{tgs[s]}",
                                     tag=f"wk_{s}")
                    dr_layer(pz2[s], w2s, h1[s])
                h2 = {}
                for s in streams:
                    h2[s] = ap_.tile([F, 2, Bh], BF16, name=f"h2{tgs[s]}",
                                     tag=f"h2_{s}")
                    if with_bias:
                        for jc in range(2):
                            nc.scalar.activation(h2[s][:, jc, :],
                                                 pz2[s][:, jc, :], AF.Tanh,
                                                 bias=b2s[:, jc:jc + 1])
                    else:
                        nc.scalar.activation(h2[s][:], pz2[s][:], AF.Tanh)
                sq2 = {}
                for s in streams:
                    sq2[s] = ap_.tile([F, 2, Bh], F8, name=f"sq2{tgs[s]}",
                                      tag=f"sq2_{s}")
                    # h2^2/4: the /4 compensates the x4-scaled M matrices
                    nc.vector.scalar_tensor_tensor(
                        sq2[s][:], h2[s][:], 0.25, h2[s][:], ALU.mult, ALU.mult)
                pd = {}
                for s in streams:
                    # constant -c2 rows land first (K=2 hi/lo matmul, depends
                    # only on consts), DR chunks then accumulate
                    pd[s] = pp.tile([F, 2, Bh], F32, name=f"pd{tgs[s]}",
                                    tag=f"wk_{s}")
                    for jc in range(2):
                        nc.tensor.matmul(
                            pd[s][:, jc, :], c2ns[:, jc, jc * F:(jc + 1) * F],
                            ones2[:, :], start=True, stop=False,
                            skip_group_check=True)
                    dr_layer(pd[s], w2tws, sq2[s], start=False)
                a = {}
                for s in streams:
                    # a = (h1 - 1) * pd'   [psum read]
                    a[s] = ap_.tile([F, 2, Bh], BF16, name=f"a{tgs[s]}",
                                    tag=f"a_{s}")
                    nc.vector.scalar_tensor_tensor(
                        a[s][:], h1[s][:], 1.0, pd[s][:], ALU.subtract,
                        ALU.mult)
                v = {}
                for s in streams:
                    # v = (h1 + 1) * a = (h1^2 - 1)(pd - c2)/4
                    v[s] = ap_.tile([F, 2, Bh], F8, name=f"v{tgs[s]}",
                                    tag=f"v{tgs[s]}")
                    nc.vector.scalar_tensor_tensor(
                        v[s][:], h1[s][:], 1.0, a[s][:], ALU.add, ALU.mult)
                if ev < 7:
                    m = mabs[c] if ev % 2 == 0 else mbas[c]
                    for s in streams:
                        dr_layer(z1s[s], m, v[s], start=False)
                return v

            def recovery(c, s, vts):
                """q_f = q0 + 16dt*(sum_A v)@W1T_p ; p_f = p0 - 8dt*(sum v)@W1T_q
                accumulated in PSUM from the stored v tiles (fp8 DoubleRow)."""
                sl = slice(s * Bh, (s + 1) * Bh)
                dtc = DT if c == 0 else -DT
                rec = pp.tile([F, 2, Bh], F32, name=f"rec_{c}_{s}", tag=f"wk_{s}")
                va = [v for ev, v in vts if ev % 2 == 0]
                for i, v in enumerate(va):
                    nc.tensor.matmul(
                        rec[:, 0, :], w1ts[:, :, F:2 * F], v[:, :, :],
                        start=(i == 0), stop=(i == len(va) - 1),
                        perf_mode=DR, skip_group_check=True)
                for i, (ev, v) in enumerate(vts):
                    nc.tensor.matmul(
                        rec[:, 1, :], w1ts[:, :, 0:F], v[:, :, :],
                        start=(i == 0), stop=(i == len(vts) - 1),
                        perf_mode=DR, skip_group_check=True)
                oq = op_.tile([F, Bh], F32, name=f"oq_{c}_{s}", tag=f"oq_{s}")
                nc.vector.scalar_tensor_tensor(
                    oq[:], rec[:, 0, :], 4.0 * dtc, q0s[:, sl],
                    ALU.mult, ALU.add)
                nc.sync.dma_start(out=outs_d["oqf" if c == 0 else "oqb"][:, sl],
                                  in_=oq[:])
                opp = op_.tile([F, Bh], F32, name=f"op_{c}_{s}", tag=f"op_{s}")
                nc.vector.scalar_tensor_tensor(
                    opp[:], rec[:, 1, :], -2.0 * dtc, p0s[:, sl],
                    ALU.mult, ALU.add)
                nc.sync.dma_start(out=outs_d["opf" if c == 0 else "opb"][:, sl],
                                  in_=opp[:])

            for c in range(2):
                z1 = [None] * NS
                vts = [[] for _ in range(NS)]   # (ev, v_tile) per stream
                for s in range(NS):
                    z1[s] = pp.tile([F, 2, Bh], F32, name=f"z1_{c}_{s}",
                                    tag=f"z1_{s}")
                    init_z1(z1[s], s)
                    if c == 1:
                        # reuse chain0's shared first eval: apply the stored
                        # v_A1 transition with this chain's M_AB
                        vts[s].append((0, va1_keep[s]))
                        dr_layer(z1[s], mabs[1], va1_keep[s], start=False)
                ev0 = 1 if c == 1 else 0
                for ev in range(ev0, 8):
                    vs = emit_eval_pair(c, ev, z1, list(range(NS)))
                    for s in range(NS):
                        vts[s].append((ev, vs[s]))
                        if c == 0 and ev == 0:
                            va1_keep[s] = vs[s]
                for s in range(NS):
                    recovery(c, s, vts[s])

    nc.finalize()
    return nc


_NC_CACHE = {}


def _get_nc(with_bias):
    key = ("nc", with_bias)
    if key not in _NC_CACHE:
        _NC_CACHE[key] = _build_program(with_bias)
    return _NC_CACHE[key]


def _blk(w, dtype):
    """[256, 256] -> [128, 2, 256] with blk[p, kc, m] = w[kc*128 + p, m]."""
    return np.ascontiguousarray(
        np.asarray(w, np.float32).reshape(2, F, 2 * F).transpose(1, 0, 2)
    ).astype(dtype)


def _col2(v):
    """[256] -> [128, 2] with out[p, jc] = v[jc*128 + p]."""
    return np.ascontiguousarray(np.asarray(v, np.float32).reshape(2, F).T)


def _prepare_in_maps(x, W1, b1, W2, b2, Wout):
    x = np.asarray(x, np.float32)
    W1 = np.asarray(W1, np.float32)
    W2 = np.asarray(W2, np.float32)
    wout = np.asarray(Wout, np.float32).reshape(-1)
    b1 = np.asarray(b1, np.float32).reshape(-1)
    b2 = np.asarray(b2, np.float32).reshape(-1)
    with_bias = bool(b1.any() or b2.any())

    q_mid = x[:, MID, :]                       # [B, F]
    p_mid = q_mid - x[:, MID - 1, :]
    qt = np.ascontiguousarray(q_mid.T)         # [F, B]
    pt = np.ascontiguousarray(p_mid.T)

    w2tw = (W2.T * wout[:, None]).astype(np.float32)  # [j,i] = wout[j]*W2[i,j]
    w2tw8 = _blk(w2tw, ml_dtypes.float8_e4m3)
    # c2 must match the fp8 weights actually used in the matmul; shipped as
    # two bf16 constant rows (hi + residual) fed via a K=2 ones-matmul
    c2 = w2tw8.astype(np.float32).transpose(1, 0, 2).reshape(2 * F, 2 * F).sum(axis=0)
    c2n = np.zeros((2, 2, 2 * F), np.float32)
    for jc in range(2):
        blkv = -0.25 * c2[jc * F:(jc + 1) * F]
        hi = blkv.astype(ml_dtypes.bfloat16).astype(np.float32)
        c2n[0, jc, jc * F:(jc + 1) * F] = hi
        c2n[1, jc, jc * F:(jc + 1) * F] = blkv - hi

    W1q, W1p = W1[:F, :], W1[F:, :]
    W1T = np.ascontiguousarray(W1.T)
    W1T_q, W1T_p = W1T[:, :F], W1T[:, F:]
    ms = {}
    for c, dtc in ((0, DT), (1, -DT)):
        ms[f"mab{c}"] = _blk(4.0 * (dtc * (W1T_p @ W1q) - 0.5 * dtc * (W1T_q @ W1p)),
                             ml_dtypes.float8_e4m3)
        ms[f"mba{c}"] = _blk(4.0 * (-0.5 * dtc * (W1T_q @ W1p)),
                             ml_dtypes.float8_e4m3)

    shared = {
        "w1": _blk(W1, ml_dtypes.bfloat16),
        "w2": _blk(W2, ml_dtypes.float8_e4m3),
        "w2tw": w2tw8,
        "w1t8": _blk(W1T, ml_dtypes.float8_e4m3),
        "c2n": c2n.astype(ml_dtypes.bfloat16),
        "b1c": _col2(b1), "b2c": _col2(b2),
        **ms,
    }
    in_maps = []
    for core in range(N_CORES):
        sl = slice(core * Bc, (core + 1) * Bc)
        m = dict(shared)
        m["qt"] = np.ascontiguousarray(qt[:, sl])
        m["pt"] = np.ascontiguousarray(pt[:, sl])
        s0 = np.empty((F, 2, Bc), np.float32)
        s0[:, 0, :] = m["qt"]
        s0[:, 1, :] = m["pt"]
        m["s0"] = s0.astype(ml_dtypes.bfloat16)
        in_maps.append(m)
    return in_maps, q_mid, p_mid, with_bias


def _assemble(results, q_mid, p_mid):
    out = np.empty((B, 6 * F), np.float32)
    out[:, 2 * F:3 * F] = q_mid
    out[:, 3 * F:4 * F] = p_mid
    for core in range(N_CORES):
        sl = slice(core * Bc, (core + 1) * Bc)
        r = results[core]
        out[sl, 0:F] = r["oqb"].T
        out[sl, F:2 * F] = r["opb"].T
        out[sl, 4 * F:5 * F] = r["oqf"].T
        out[sl, 5 * F:6 * F] = r["opf"].T
    return out


def run(trace=False, **inputs):
    """Full pipeline; returns (output, BassKernelResults)."""
    in_maps, q_mid, p_mid, with_bias = _prepare_in_maps(**inputs)
    nc = _get_nc(with_bias)
    res = run_bass_kernel_spmd(nc, in_maps, list(range(N_CORES)), trace=trace)
    return _assemble(res.results, q_mid, p_mid), res


def kernel(**inputs) -> np.ndarray:
    out, _ = run(trace=False, **inputs)
    return out


if __name__ == "__main__" and "--simcheck" in sys.argv:
    from concourse.bass_interp import CoreSim

    d = np.load(os.path.join(os.path.dirname(os.path.abspath(__file__)),
                             "_ref_data.npz"))
    inputs = {k: d[k] for k in ("x", "W1", "b1", "W2", "b2", "Wout")}
    expected = d["expected"]
    in_maps, q_mid, p_mid, with_bias = _prepare_in_maps(**inputs)
    nc = _get_nc(with_bias)
    sim = CoreSim(nc)
    for k, val in in_maps[0].items():
        sim.tensor(k)[:] = val
    sim.simulate()
    results = [{k: np.array(sim.tensor(k)) for k in ("oqf", "opf", "oqb", "opb")}]
    exp = expected[:Bc]
    out = np.empty((Bc, 6 * F), np.float32)
    out[:, 2 * F:3 * F] = q_mid[:Bc]
    out[:, 3 * F:4 * F] = p_mid[:Bc]
    r = results[0]
    out[:, 0:F] = r["oqb"].T
    out[:, F:2 * F] = r["opb"].T
    out[:, 4 * F:5 * F] = r["oqf"].T
    out[:, 5 * F:6 * F] = r["opf"].T
    err = np.abs(out - exp.astype(np.float64))
    rel = err.max() / np.abs(exp).max()
    print(f"simcheck: absmax {err.max():.3e}  rel {rel:.6e}")
